# revision 1
# baseline (speedup 1.0000x reference)
"""BitNet transformer block on 8 Trainium2 NeuronCores (Megatron tensor-parallel).

Self-contained: builds one SPMD Bass/Tile program, shards inputs on host,
runs via run_bass_kernel_spmd, gathers the output.

Sharding (per core c of 8):
  - wq/wk/wv column-parallel: output rows c*256:(c+1)*256 (= heads 2c, 2c+1)
  - wg/wu column-parallel: output rows c*1024:(c+1)*1024
  - wo/wd row-parallel: input cols c*256 / c*1024 slices
  - norms: row-parallel (each core normalizes its 256 rows, AllGathers the
    quantized activations)
All weight shards are host-transposed to K-major [k, out] layouts so SBUF
tiles have the contraction dim on partitions.

Numerics: quantized activations (ints in [-127,127]) and ternary weights are
exact in bf16; matmuls accumulate in fp32 PSUM, so every BitNet matmul is
exact integer arithmetic. Rounding uses the fp32 magic-constant trick
(+1.5*2^23), which matches jnp.round's ties-to-even. Only the attention
scores/probabilities/value path carries bf16 rounding noise.
"""

import os

import numpy as np
import ml_dtypes

import concourse.bacc as bacc
import concourse.mybir as mybir
import concourse.tile as tile
from concourse.bass_utils import run_bass_kernel_spmd

F32 = mybir.dt.float32
BF16 = mybir.dt.bfloat16
AF = mybir.ActivationFunctionType
ALU = mybir.AluOpType
AX = mybir.AxisListType

NCORES = 8
# BITNET_S / BITNET_MLP env overrides exist only for scaled-down simulator
# testing; the graded problem shape is the default.
B, S, D, H, MLP = 2, int(os.environ.get("BITNET_S", "1024")), 2048, 16, \
    int(os.environ.get("BITNET_MLP", "8192"))
HD = 128
R = B * S                 # 2048 rows total
RL = R // NCORES          # 256 rows per core (row shard)
OQ = D // NCORES          # 256 qkv out cols per core (2 heads)
OM = MLP // NCORES        # 1024 mlp cols per core
P = 128
KT = D // P               # 16 feature chunks
RT = R // P               # 16 row tiles
LT = RL // P              # 2 local row tiles
ST = S // P               # 8 seq tiles per batch
MAGIC = 12582912.0        # 1.5 * 2**23: fp32 round-to-nearest-even magic
INV_SQRT_HD = 1.0 / float(np.sqrt(HD))

_CACHED_NC = None


def _quant(nc, sp, src_ap, qscale_ap, out_bf_ap, wclip=False, tag="qtmp"):
    """out_bf = round(src * qscale) as bf16 (clip to [-1,1] if wclip).

    fp32 +MAGIC rounds to integer (RNE); ACT subtracts MAGIC and casts to
    bf16 (small ints are exact in bf16).
    """
    F = src_ap.shape[1]
    CH = min(F, 1024)  # bound the fp32 scratch to 4KB/partition
    for c0 in range(0, F, CH):
        tmp = sp.tile([src_ap.shape[0], CH], F32, tag=tag, name=tag)
        if qscale_ap is None:
            nc.vector.tensor_scalar(
                tmp[:], src_ap[:, c0:c0 + CH], MAGIC, None, op0=ALU.add
            )
        else:
            nc.vector.tensor_scalar(
                tmp[:], src_ap[:, c0:c0 + CH], qscale_ap, MAGIC,
                op0=ALU.mult, op1=ALU.add,
            )
        nc.scalar.activation(
            out_bf_ap[:, c0:c0 + CH], tmp[:], AF.Copy, bias=-MAGIC, scale=1.0
        )
    if wclip:
        nc.vector.tensor_scalar(
            out_bf_ap, out_bf_ap, 1.0, -1.0, op0=ALU.min, op1=ALU.max
        )


def _rms_quant_rows(nc, sp, ps_dummy, src_tile, nw_tile, as_out_ap, aq_out_ap):
    """rmsnorm + abs-max + int8-grid quantize for one [128, D] row tile.

    Writes a_scale (max|h|+1e-8, h = src/rms*nw) to as_out_ap [128,1] and
    the quantized bf16 ints to aq_out_ap. Mutates src_tile in place
    (src *= nw).
    """
    sqd = ps_dummy.tile([P, D], F32, tag="sqd")
    ssq = sp.tile([P, 1], F32, tag="ssq")
    nc.scalar.activation(sqd[:], src_tile[:], AF.Square, accum_out=ssq[:])
    rms = sp.tile([P, 1], F32, tag="rms")
    nc.vector.tensor_scalar(
        rms[:], ssq[:], 1.0 / D, 1e-6, op0=ALU.mult, op1=ALU.add
    )
    nc.scalar.activation(rms[:], rms[:], AF.Sqrt)
    rinv = sp.tile([P, 1], F32, tag="rinv")
    nc.vector.reciprocal(rinv[:], rms[:])
    nc.vector.tensor_tensor(src_tile[:], src_tile[:], nw_tile[:], op=ALU.mult)
    amax = sp.tile([P, 1], F32, tag="amax")
    nc.vector.tensor_reduce(
        amax[:], src_tile[:], op=ALU.max, axis=AX.X, apply_absolute_value=True
    )
    nc.vector.tensor_scalar(
        as_out_ap, amax[:], rinv[:], 1e-8, op0=ALU.mult, op1=ALU.add
    )
    inva = sp.tile([P, 1], F32, tag="inva")
    nc.vector.reciprocal(inva[:], as_out_ap)
    qs = sp.tile([P, 1], F32, tag="qs")
    nc.vector.tensor_scalar(
        qs[:], inva[:], rinv[:], 127.0, op0=ALU.mult, op1=ALU.mult
    )
    _quant(nc, sp, src_tile[:], qs[:, 0:1], aq_out_ap)


def build_program():
    nc = bacc.Bacc(
        "TRN2",
        target_bir_lowering=False,
        debug=False,
        enable_asserts=True,
        num_devices=NCORES,
    )
    rg = [list(range(NCORES))]

    # ---------------- I/O ----------------
    x_rows = nc.dram_tensor("x_rows", [RL, D], F32, kind="ExternalInput").ap()
    wqkvT = nc.dram_tensor("wqkvT", [D, 3 * OQ], F32, kind="ExternalInput").ap()
    woT = nc.dram_tensor("woT", [OQ, D], F32, kind="ExternalInput").ap()
    wguT = nc.dram_tensor("wguT", [D, 2 * OM], F32, kind="ExternalInput").ap()
    wdT = nc.dram_tensor("wdT", [OM, D], F32, kind="ExternalInput").ap()
    norm1_w = nc.dram_tensor("norm1_w", [1, D], F32, kind="ExternalInput").ap()
    norm2_w = nc.dram_tensor("norm2_w", [1, D], F32, kind="ExternalInput").ap()
    ident_b = nc.dram_tensor("ident_b", [P, P], BF16, kind="ExternalInput").ap()
    causal = nc.dram_tensor("causal", [P, P], F32, kind="ExternalInput").ap()
    wcnt_inv = nc.dram_tensor("wcnt_inv", [1, 8], F32, kind="ExternalInput").ap()
    out_d = nc.dram_tensor("out", [RL, D], F32, kind="ExternalOutput").ap()

    with tile.TileContext(nc) as tc, \
         tc.tile_pool(name="persist", bufs=1) as pp, \
         tc.tile_pool(name="dram", bufs=1, space="DRAM") as dp:

        # ---------------- constants ----------------
        ident = pp.tile([P, P], BF16, tag="ident")
        nc.sync.dma_start(ident[:], ident_b)
        mask = pp.tile([P, P], F32, tag="mask")
        nc.sync.dma_start(mask[:], causal)
        ones8 = pp.tile([P, 8], F32, tag="ones8")
        nc.vector.memset(ones8[:], 1.0)
        wci = pp.tile([1, 8], F32, tag="wci")
        nc.sync.dma_start(wci[:], wcnt_inv)

        # persistent scale tiles (small)
        as1g = pp.tile([P, RT], F32, tag="as1g")
        cq = pp.tile([P, RT], F32, tag="cq")
        scv = pp.tile([P, RT], F32, tag="scv")
        aso = pp.tile([P, RT], F32, tag="aso")

        # collective DRAM buffers
        ag1_in = dp.tile([RL, D], BF16, tag="ag1_in")
        ag1_out = dp.tile([R, D], BF16, tag="ag1_out")
        ag1s_in = dp.tile([RL, 1], F32, tag="ag1s_in")
        ag1s_out = dp.tile([R, 1], F32, tag="ag1s_out")
        x1_d = dp.tile([RL, D], F32, tag="x1_d")

        # =========================================================
        # Mega-pool 1: phases W,1,2,3 + o-proj (frees before MLP)
        # =========================================================
        with tc.tile_pool(name="mp1", bufs=1) as m1, \
             tc.tile_pool(name="scr1", bufs=2) as s1:

            # ---- Phase 1: local rmsnorm1 + quant + AllGather ----
            nw1 = m1.tile([P, D], F32, tag="nw1")
            nw1r = m1.tile([1, D], F32, tag="nw1r")
            nc.sync.dma_start(nw1r[:], norm1_w)
            nc.gpsimd.partition_broadcast(nw1[:], nw1r[0:1, :])

            with tc.tile_pool(name="ps0", bufs=1, space="PSUM") as ps0:
                for lt in range(LT):
                    xt = s1.tile([P, D], F32, tag="xt")
                    nc.sync.dma_start(xt[:], x_rows[lt * P:(lt + 1) * P, :])
                    as_l = s1.tile([P, 1], F32, tag="as_l")
                    aq = s1.tile([P, D], BF16, tag="aq")
                    _rms_quant_rows(nc, s1, ps0, xt, nw1, as_l[:, 0:1], aq[:])
                    nc.sync.dma_start(ag1s_in[lt * P:(lt + 1) * P, :], as_l[:])
                    nc.sync.dma_start(ag1_in[lt * P:(lt + 1) * P, :], aq[:])
            nc.gpsimd.collective_compute(
                "AllGather", ALU.bypass, replica_groups=rg,
                ins=[ag1_in.opt()], outs=[ag1_out.opt()],
            )
            nc.gpsimd.collective_compute(
                "AllGather", ALU.bypass, replica_groups=rg,
                ins=[ag1s_in.opt()], outs=[ag1s_out.opt()],
            )
            nc.sync.dma_start(
                as1g[:], ag1s_out.rearrange("(t p) o -> p (t o)", p=P)
            )

            # ---- Phase W: weight scales ----
            W_TILES = {
                0: (wqkvT, KT, 0, OQ),
                1: (wqkvT, KT, OQ, 2 * OQ),
                2: (wqkvT, KT, 2 * OQ, 3 * OQ),
                3: (woT, OQ // P, 0, D),
                4: (wguT, KT, 0, OM),
                5: (wguT, KT, OM, 2 * OM),
                6: (wdT, OM // P, 0, D),
            }
            svec = pp.tile([P, 8], F32, tag="svec")
            nc.vector.memset(svec[:], 0.0)
            accs = {
                slot: m1.tile([P, nt], F32, tag=f"acc{slot}", name=f"acc{slot}")
                for slot, (_, nt, _, _) in W_TILES.items()
            }
            by_tensor = [(wqkvT, KT, [0, 1, 2]), (woT, OQ // P, [3]),
                         (wguT, KT, [4, 5]), (wdT, OM // P, [6])]
            for ap, nt, slots in by_tensor:
                for i in range(nt):
                    wtile = s1.tile([P, ap.shape[1]], F32, tag="wf")
                    nc.sync.dma_start(wtile[:], ap[i * P:(i + 1) * P, :])
                    for slot in slots:
                        _, _, lo, hi = W_TILES[slot]
                        nc.vector.tensor_reduce(
                            accs[slot][:, i:i + 1], wtile[:, lo:hi], op=ALU.add,
                            axis=AX.X, apply_absolute_value=True,
                        )
            for slot, (_, nt, _, _) in W_TILES.items():
                nc.vector.tensor_reduce(
                    svec[:, slot:slot + 1], accs[slot][:, 0:nt], op=ALU.add,
                    axis=AX.X,
                )
            with tc.tile_pool(name="psw", bufs=1, space="PSUM") as psw:
                tot_ps = psw.tile([8, 8], F32)
                nc.tensor.matmul(tot_ps[:], ones8[:, 0:8], svec[:, 0:8],
                                 start=True, stop=True)
                wsum_row = pp.tile([1, 8], F32, tag="wsum_row")
                nc.vector.tensor_copy(wsum_row[:], tot_ps[0:1, :])
            ws_in = dp.tile([1, 8], F32, tag="ws_in")
            ws_out = dp.tile([1, 8], F32, tag="ws_out")
            nc.sync.dma_start(ws_in[:], wsum_row[:])
            nc.gpsimd.collective_compute(
                "AllReduce", ALU.add, replica_groups=rg,
                ins=[ws_in.opt()], outs=[ws_out.opt()],
            )
            wsg_row = pp.tile([1, 8], F32, tag="wsg_row")
            nc.sync.dma_start(wsg_row[:], ws_out[:])
            ws_row = pp.tile([1, 8], F32, tag="ws_row")
            nc.vector.tensor_tensor(ws_row[:], wsg_row[:], wci[0:1, :], op=ALU.mult)
            nc.vector.tensor_scalar(ws_row[:], ws_row[:], 1e-8, None, op0=ALU.add)
            inv_row = pp.tile([1, 8], F32, tag="inv_row")
            nc.vector.reciprocal(inv_row[:], ws_row[:])
            wsb, invb = {}, {}
            for slot in range(7):
                wsb[slot] = pp.tile([P, 1], F32, tag=f"wsb{slot}", name=f"wsb{slot}")
                nc.gpsimd.partition_broadcast(
                    wsb[slot][:], ws_row[0:1, slot:slot + 1]
                )
                invb[slot] = pp.tile([P, 1], F32, tag=f"invb{slot}", name=f"invb{slot}")
                nc.gpsimd.partition_broadcast(
                    invb[slot][:], inv_row[0:1, slot:slot + 1]
                )

            # cq = as1 * ws_q*ws_k/(127^2 sqrt(HD)); scv = as1 * ws_v/127
            pw = pp.tile([P, 1], F32, tag="pw")
            nc.vector.tensor_tensor(pw[:], wsb[0][:], wsb[1][:], op=ALU.mult)
            nc.vector.tensor_scalar(
                pw[:], pw[:], INV_SQRT_HD / (127.0 * 127.0), None, op0=ALU.mult
            )
            nc.vector.tensor_scalar(cq[:], as1g[:], pw[:, 0:1], None, op0=ALU.mult)
            nc.vector.tensor_scalar(
                scv[:], as1g[:], wsb[2][:, 0:1], 1.0 / 127.0,
                op0=ALU.mult, op1=ALU.mult,
            )
            dk_row = m1.tile([1, R], F32, tag="nw1r")  # reuse nw1r slot
            nc.sync.dma_start(dk_row[:], ag1s_out.rearrange("r o -> o r"))
            dkb = m1.tile([P, R], F32, tag="dkb")
            nc.gpsimd.partition_broadcast(dkb[:], dk_row[0:1, :])

            # ---- Phase 2: wqkv quant + QKV matmuls ----
            wqkv_q = [m1.tile([P, 3 * OQ], BF16, tag=f"wqkv_q{k}", name=f"wqkv_q{k}")
                      for k in range(KT)]
            for k in range(KT):
                wt = s1.tile([P, 3 * OQ], F32, tag="wf")
                nc.sync.dma_start(wt[:], wqkvT[k * P:(k + 1) * P, :])
                for r, slot in ((0, 0), (1, 1), (2, 2)):
                    _quant(
                        nc, s1, wt[:, r * OQ:(r + 1) * OQ], invb[slot][:, 0:1],
                        wqkv_q[k][:, r * OQ:(r + 1) * OQ], wclip=True, tag="qtmp",
                    )

            qkT = [m1.tile([P, R], BF16, tag=f"qkT{ot}", name=f"qkT{ot}") for ot in range(4)]
            v_deq = {
                (b, hl, j): m1.tile([P, P], BF16, tag=f"vd{b}_{hl}_{j}", name=f"vd{b}_{hl}_{j}")
                for b in range(B) for hl in range(2) for j in range(ST)
            }
            with tc.tile_pool(name="ps2", bufs=3, space="PSUM") as ps2:
                for t in range(RT):
                    a1b = []
                    for kb in range(KT):
                        blk = s1.tile([P, P], BF16, tag=f"a1b{kb}", name=f"a1b{kb}")
                        nc.sync.dma_start(
                            blk[:],
                            ag1_out[t * P:(t + 1) * P, kb * P:(kb + 1) * P],
                            transpose=True,
                        )
                        a1b.append(blk)
                    ps = ps2.tile([P, 3 * OQ], F32, tag="psqkv")
                    for kb in range(KT):
                        nc.tensor.matmul(
                            ps[:, 0:512], a1b[kb][:], wqkv_q[kb][:, 0:512],
                            start=(kb == 0), stop=(kb == KT - 1),
                        )
                        nc.tensor.matmul(
                            ps[:, 512:768], a1b[kb][:], wqkv_q[kb][:, 512:768],
                            start=(kb == 0), stop=(kb == KT - 1),
                        )
                    qkstage = s1.tile([P, 4 * P], BF16, tag="qkstage")
                    nc.vector.tensor_copy(qkstage[:], ps[:, 0:512])
                    for ot in range(4):
                        nc.sync.dma_start(
                            qkT[ot][:, t * P:(t + 1) * P],
                            qkstage[:, ot * P:(ot + 1) * P],
                            transpose=True,
                        )
                    b, j = divmod(t, ST)
                    for hl in range(2):
                        nc.vector.tensor_scalar(
                            v_deq[(b, hl, j)][:],
                            ps[:, 512 + hl * P: 512 + (hl + 1) * P],
                            scv[:, t:t + 1], None, op0=ALU.mult,
                        )

            # ---- Phase 3: attention ----
            attn_sb = [m1.tile([P, 2 * P], F32, tag=f"attn{t}", name=f"attn{t}")
                       for t in range(RT)]
            with tc.tile_pool(name="ps3s", bufs=2, space="PSUM") as ps3s, \
                 tc.tile_pool(name="ps3a", bufs=4, space="PSUM") as ps3a:
                for b in range(B):
                    for i in range(ST):
                        tg = b * ST + i
                        L = (i + 1) * P
                        for hl in range(2):
                            q_ot, k_ot = hl, 2 + hl
                            Spp = ps3s.tile([P, S], F32, tag="Sp")
                            lhsT = qkT[q_ot][:, b * S + i * P: b * S + (i + 1) * P]
                            for jc in range((L + 511) // 512):
                                n0 = jc * 512
                                n1 = min(L, n0 + 512)
                                nc.tensor.matmul(
                                    Spp[:, n0:n1], lhsT,
                                    qkT[k_ot][:, b * S + n0: b * S + n1],
                                    start=True, stop=True,
                                )
                            S1 = s1.tile([P, S], F32, tag="S1")
                            nc.vector.scalar_tensor_tensor(
                                S1[:, 0:L], Spp[:, 0:L], cq[:, tg:tg + 1],
                                dkb[:, b * S: b * S + L],
                                op0=ALU.mult, op1=ALU.mult,
                            )
                            nc.vector.tensor_tensor(
                                S1[:, i * P:L], S1[:, i * P:L], mask[:], op=ALU.add
                            )
                            negmx = s1.tile([P, 1], F32, tag="negmx")
                            nc.vector.tensor_reduce(
                                negmx[:], S1[:, 0:L], op=ALU.max, axis=AX.X,
                                negate=True,
                            )
                            esum = s1.tile([P, 1], F32, tag="esum")
                            # exp in place over S1 (saves a [128,1024] slot)
                            nc.scalar.activation(
                                S1[:, 0:L], S1[:, 0:L], AF.Exp,
                                bias=negmx[:, 0:1], scale=1.0, accum_out=esum[:],
                            )
                            erec = s1.tile([P, 1], F32, tag="erec")
                            nc.vector.reciprocal(erec[:], esum[:])
                            Pb = s1.tile([P, S], BF16, tag="Pb")
                            nc.vector.tensor_scalar(
                                Pb[:, 0:L], S1[:, 0:L], erec[:, 0:1], None,
                                op0=ALU.mult,
                            )
                            att = ps3a.tile([P, P], F32, tag="att")
                            for j in range(i + 1):
                                pts = s1.tile([P, P], BF16, tag="pts", bufs=4)
                                nc.sync.dma_start(
                                    pts[:], Pb[:, j * P:(j + 1) * P],
                                    transpose=True,
                                )
                                nc.tensor.matmul(
                                    att[:], pts[:], v_deq[(b, hl, j)][:],
                                    start=(j == 0), stop=(j == i),
                                )
                            nc.vector.tensor_copy(
                                attn_sb[tg][:, hl * P:(hl + 1) * P], att[:]
                            )
                        nc.vector.tensor_reduce(
                            aso[:, tg:tg + 1], attn_sb[tg][:], op=ALU.max,
                            axis=AX.X, apply_absolute_value=True,
                        )

            # a_scale_o: AR-max (global) + RS-max (local)
            aso_in = dp.tile([R, 1], F32, tag="aso_in")
            aso_g = dp.tile([R, 1], F32, tag="aso_g")
            aso_l = dp.tile([RL, 1], F32, tag="aso_l")
            nc.sync.dma_start(
                aso_in.rearrange("(t p) o -> p (t o)", p=P), aso[:]
            )
            nc.gpsimd.collective_compute(
                "AllReduce", ALU.max, replica_groups=rg,
                ins=[aso_in.opt()], outs=[aso_g.opt()],
            )
            nc.gpsimd.collective_compute(
                "ReduceScatter", ALU.max, replica_groups=rg,
                ins=[aso_in.opt()], outs=[aso_l.opt()],
            )
            asog = pp.tile([P, RT], F32, tag="asog")
            nc.sync.dma_start(
                asog[:], aso_g.rearrange("(t p) o -> p (t o)", p=P)
            )
            nc.vector.tensor_scalar(asog[:], asog[:], 1e-8, None, op0=ALU.add)
            asol = pp.tile([P, LT], F32, tag="asol")
            nc.sync.dma_start(
                asol[:], aso_l.rearrange("(t p) o -> p (t o)", p=P)
            )
            nc.vector.tensor_scalar(asol[:], asol[:], 1e-8, None, op0=ALU.add)
            qso = pp.tile([P, RT], F32, tag="qso")
            nc.vector.reciprocal(qso[:], asog[:])
            nc.vector.tensor_scalar(qso[:], qso[:], 127.0, None, op0=ALU.mult)
            sc_ol = pp.tile([P, LT], F32, tag="sc_ol")
            nc.vector.tensor_scalar(
                sc_ol[:], asol[:], wsb[3][:, 0:1], 1.0 / 127.0,
                op0=ALU.mult, op1=ALU.mult,
            )

            # quantize attn_out -> a_oT; quantize woT
            a_oT = [m1.tile([P, R], BF16, tag=f"a_oT{kk}", name=f"a_oT{kk}")
                    for kk in range(OQ // P)]
            for t in range(RT):
                a_qo = s1.tile([P, 2 * P], BF16, tag="a_qo")
                _quant(nc, s1, attn_sb[t][:], qso[:, t:t + 1], a_qo[:],
                       tag="qotmp")
                for kk in range(OQ // P):
                    nc.sync.dma_start(
                        a_oT[kk][:, t * P:(t + 1) * P],
                        a_qo[:, kk * P:(kk + 1) * P], transpose=True,
                    )
            wo_q = [m1.tile([P, D], BF16, tag=f"wo_q{kk}", name=f"wo_q{kk}")
                    for kk in range(OQ // P)]
            for kk in range(OQ // P):
                wt = s1.tile([P, D], F32, tag="wf")
                nc.sync.dma_start(wt[:], woT[kk * P:(kk + 1) * P, :])
                _quant(nc, s1, wt[:], invb[3][:, 0:1], wo_q[kk][:],
                       wclip=True, tag="qtmp")

            # o-proj int partials, 2 col chunks, RS each
            rs1_out = []
            with tc.tile_pool(name="ps5", bufs=4, space="PSUM") as ps5:
                for oc in range(2):
                    rs_in = dp.tile([R, D // 2], F32, tag=f"rs1i{oc}")
                    rs_out = dp.tile([RL, D // 2], F32, tag=f"rs1o{oc}")
                    rs1_out.append(rs_out)
                    for t in range(RT):
                        pso = ps5.tile([P, D // 2], F32, tag="ops")
                        for kk in range(OQ // P):
                            lhsT = a_oT[kk][:, t * P:(t + 1) * P]
                            for n in range(2):
                                c0 = oc * (D // 2) + n * 512
                                nc.tensor.matmul(
                                    pso[:, n * 512:(n + 1) * 512], lhsT,
                                    wo_q[kk][:, c0:c0 + 512],
                                    start=(kk == 0), stop=(kk == OQ // P - 1),
                                )
                        osb = s1.tile([P, D // 2], F32, tag="xt")  # reuse xt slot
                        nc.vector.tensor_copy(osb[:], pso[:])
                        nc.sync.dma_start(rs_in[t * P:(t + 1) * P, :], osb[:])
                    nc.gpsimd.collective_compute(
                        "ReduceScatter", ALU.add, replica_groups=rg,
                        ins=[rs_in.opt()], outs=[rs_out.opt()],
                    )
        # mega-pool 1 frees here

        # =========================================================
        # Mega-pool 2: phases 4,5,6
        # =========================================================
        ag2_in = dp.tile([RL, D], BF16, tag="ag2_in")
        ag2_out = dp.tile([R, D], BF16, tag="ag2_out")
        ag2s_in = dp.tile([RL, 1], F32, tag="ag2s_in")
        ag2s_out = dp.tile([R, 1], F32, tag="ag2s_out")

        # ---- Phase 4: residual + rmsnorm2 + quant + AG ----
        # own pool scope: frees before the big MLP arrays allocate
        with tc.tile_pool(name="p4", bufs=1) as p4, \
             tc.tile_pool(name="s4", bufs=2) as s4, \
             tc.tile_pool(name="ps40", bufs=1, space="PSUM") as ps40:
            nw2 = p4.tile([P, D], F32, tag="nw2")
            nw2r = p4.tile([1, D], F32, tag="nw2r")
            nc.sync.dma_start(nw2r[:], norm2_w)
            nc.gpsimd.partition_broadcast(nw2[:], nw2r[0:1, :])
            for lt in range(LT):
                x1t = s4.tile([P, D], F32, tag="x1t", bufs=1)
                xr = p4.tile([P, D], F32, tag="xr")
                nc.sync.dma_start(xr[:], x_rows[lt * P:(lt + 1) * P, :])
                for oc in range(2):
                    ysb = s4.tile([P, D // 2], F32, tag="c1024")
                    nc.sync.dma_start(
                        ysb[:], rs1_out[oc][lt * P:(lt + 1) * P, :]
                    )
                    nc.vector.scalar_tensor_tensor(
                        x1t[:, oc * (D // 2):(oc + 1) * (D // 2)],
                        ysb[:], sc_ol[:, lt:lt + 1],
                        xr[:, oc * (D // 2):(oc + 1) * (D // 2)],
                        op0=ALU.mult, op1=ALU.add,
                    )
                nc.sync.dma_start(x1_d[lt * P:(lt + 1) * P, :], x1t[:])
                as_l = p4.tile([P, 1], F32, tag="as_l2")
                aq2 = p4.tile([P, D], BF16, tag="aq2")
                _rms_quant_rows(nc, s4, ps40, x1t, nw2, as_l[:, 0:1], aq2[:])
                nc.sync.dma_start(ag2s_in[lt * P:(lt + 1) * P, :], as_l[:])
                nc.sync.dma_start(ag2_in[lt * P:(lt + 1) * P, :], aq2[:])

        nc.gpsimd.collective_compute(
            "AllGather", ALU.bypass, replica_groups=rg,
            ins=[ag2_in.opt()], outs=[ag2_out.opt()],
        )
        nc.gpsimd.collective_compute(
            "AllGather", ALU.bypass, replica_groups=rg,
            ins=[ag2s_in.opt()], outs=[ag2s_out.opt()],
        )

        with tc.tile_pool(name="mp2", bufs=1) as m2, \
             tc.tile_pool(name="scr2", bufs=2) as s2:
            as2g = pp.tile([P, RT], F32, tag="as2g")
            nc.sync.dma_start(
                as2g[:], ag2s_out.rearrange("(t p) o -> p (t o)", p=P)
            )
            sc_g = pp.tile([P, RT], F32, tag="sc_g")
            nc.vector.tensor_scalar(
                sc_g[:], as2g[:], wsb[4][:, 0:1], 1.0 / 127.0,
                op0=ALU.mult, op1=ALU.mult,
            )
            sc_u = pp.tile([P, RT], F32, tag="sc_u")
            nc.vector.tensor_scalar(
                sc_u[:], as2g[:], wsb[5][:, 0:1], 1.0 / 127.0,
                op0=ALU.mult, op1=ALU.mult,
            )

            # ---- Phase 5: wgu quant, gate/up, m, group AR-max, mT ----
            wgu_q = [m2.tile([P, 2 * OM], BF16, tag=f"wgu_q{k}", name=f"wgu_q{k}")
                     for k in range(KT)]
            for k in range(KT):
                for half in range(2):
                    wt = s2.tile([P, OM], F32, tag="wf2")
                    nc.sync.dma_start(
                        wt[:], wguT[k * P:(k + 1) * P, half * OM:(half + 1) * OM]
                    )
                    _quant(
                        nc, s2, wt[:], invb[4 + half][:, 0:1],
                        wgu_q[k][:, half * OM:(half + 1) * OM], wclip=True,
                        tag="qtmp",
                    )

            NG = 4
            GT = RT // NG
            asm = pp.tile([P, RT], F32, tag="asm")
            asm_in = [dp.tile([GT * P, 1], F32, tag=f"asmi{g}", name=f"asmi{g}") for g in range(NG)]
            asm_go = [dp.tile([GT * P, 1], F32, tag=f"asmo{g}", name=f"asmo{g}") for g in range(NG)]
            asm_rsin = dp.tile([R, 1], F32, tag="asm_rsin")
            asm_lout = dp.tile([RL, 1], F32, tag="asm_lout")
            asmg = pp.tile([P, RT], F32, tag="asmg")
            qsm = pp.tile([P, RT], F32, tag="qsm")
            m_tiles = [m2.tile([P, OM], F32, tag=f"m{t % 5}", name=f"m{t % 5}") for t in range(RT)]
            mT = [m2.tile([P, R], BF16, tag=f"mT{kb}", name=f"mT{kb}") for kb in range(OM // P)]

            with tc.tile_pool(name="ps6", bufs=2, space="PSUM") as ps6:
                for g in range(NG):
                    for tl in range(GT):
                        t = g * GT + tl
                        a2b = []
                        for kb in range(KT):
                            blk = s2.tile([P, P], BF16, tag=f"a2b{kb}", name=f"a2b{kb}")
                            nc.sync.dma_start(
                                blk[:],
                                ag2_out[t * P:(t + 1) * P, kb * P:(kb + 1) * P],
                                transpose=True,
                            )
                            a2b.append(blk)
                        psg = ps6.tile([P, 2 * OM], F32, tag="psg")
                        for kb in range(KT):
                            for n in range(2 * OM // 512):
                                nc.tensor.matmul(
                                    psg[:, n * 512:(n + 1) * 512], a2b[kb][:],
                                    wgu_q[kb][:, n * 512:(n + 1) * 512],
                                    start=(kb == 0), stop=(kb == KT - 1),
                                )
                        # silu(g_deq) = g_deq * sigmoid(g_deq), g_deq = g*sc_g
                        sig = s2.tile([P, OM], F32, tag="c1024")
                        nc.scalar.activation(
                            sig[:], psg[:, 0:OM], AF.Sigmoid, scale=sc_g[:, t:t + 1]
                        )
                        sgl = s2.tile([P, OM], F32, tag="sgl", bufs=1)
                        nc.vector.scalar_tensor_tensor(
                            sgl[:], psg[:, 0:OM], sc_g[:, t:t + 1], sig[:],
                            op0=ALU.mult, op1=ALU.mult,
                        )
                        nc.vector.scalar_tensor_tensor(
                            m_tiles[t][:], psg[:, OM:2 * OM], sc_u[:, t:t + 1],
                            sgl[:], op0=ALU.mult, op1=ALU.mult,
                        )
                        nc.vector.tensor_reduce(
                            asm[:, t:t + 1], m_tiles[t][:], op=ALU.max, axis=AX.X,
                            apply_absolute_value=True,
                        )
                    nc.sync.dma_start(
                        asm_in[g].rearrange("(t p) o -> p (t o)", p=P),
                        asm[:, g * GT:(g + 1) * GT],
                    )
                    nc.sync.dma_start(
                        asm_rsin[g * GT * P:(g + 1) * GT * P, :]
                        .rearrange("(t p) o -> p (t o)", p=P),
                        asm[:, g * GT:(g + 1) * GT],
                    )
                    nc.gpsimd.collective_compute(
                        "AllReduce", ALU.max, replica_groups=rg,
                        ins=[asm_in[g].opt()], outs=[asm_go[g].opt()],
                    )
                    nc.sync.dma_start(
                        asmg[:, g * GT:(g + 1) * GT],
                        asm_go[g].rearrange("(t p) o -> p (t o)", p=P),
                    )
                    nc.vector.tensor_scalar(
                        asmg[:, g * GT:(g + 1) * GT],
                        asmg[:, g * GT:(g + 1) * GT], 1e-8, None, op0=ALU.add,
                    )
                    nc.vector.reciprocal(
                        qsm[:, g * GT:(g + 1) * GT], asmg[:, g * GT:(g + 1) * GT]
                    )
                    nc.vector.tensor_scalar(
                        qsm[:, g * GT:(g + 1) * GT], qsm[:, g * GT:(g + 1) * GT],
                        127.0, None, op0=ALU.mult,
                    )
                    for tl in range(GT):
                        t = g * GT + tl
                        m_q = s2.tile([P, OM], BF16, tag="m_q", bufs=1)
                        _quant(nc, s2, m_tiles[t][:], qsm[:, t:t + 1], m_q[:],
                               tag="qtmp")
                        for kb in range(OM // P):
                            nc.sync.dma_start(
                                mT[kb][:, t * P:(t + 1) * P],
                                m_q[:, kb * P:(kb + 1) * P], transpose=True,
                            )

            nc.gpsimd.collective_compute(
                "ReduceScatter", ALU.max, replica_groups=rg,
                ins=[asm_rsin.opt()], outs=[asm_lout.opt()],
            )
            asml = pp.tile([P, LT], F32, tag="asml")
            nc.sync.dma_start(
                asml[:], asm_lout.rearrange("(t p) o -> p (t o)", p=P)
            )
            nc.vector.tensor_scalar(asml[:], asml[:], 1e-8, None, op0=ALU.add)
            sc_dl = pp.tile([P, LT], F32, tag="sc_dl")
            nc.vector.tensor_scalar(
                sc_dl[:], asml[:], wsb[6][:, 0:1], 1.0 / 127.0,
                op0=ALU.mult, op1=ALU.mult,
            )

            # ---- Phase 6: down matmuls (wd quantized per col half), RS ----
            rs2_out = []
            wd_q = [m2.tile([P, D // 2], BF16, tag=f"wd_q{kb}", name=f"wd_q{kb}")
                    for kb in range(OM // P)]
            with tc.tile_pool(name="ps7", bufs=4, space="PSUM") as ps7:
                for oc in range(2):
                    for kb in range(OM // P):
                        wt = s2.tile([P, D // 2], F32, tag="wf2")
                        nc.sync.dma_start(
                            wt[:],
                            wdT[kb * P:(kb + 1) * P,
                                oc * (D // 2):(oc + 1) * (D // 2)],
                        )
                        _quant(nc, s2, wt[:], invb[6][:, 0:1], wd_q[kb][:],
                               wclip=True, tag="qtmp")
                    rs_in = dp.tile([R, D // 2], F32, tag=f"rs2i{oc}")
                    rs_out = dp.tile([RL, D // 2], F32, tag=f"rs2o{oc}")
                    rs2_out.append(rs_out)
                    for t in range(RT):
                        pso = ps7.tile([P, D // 2], F32, tag="dps")
                        for kb in range(OM // P):
                            lhsT = mT[kb][:, t * P:(t + 1) * P]
                            for n in range(2):
                                nc.tensor.matmul(
                                    pso[:, n * 512:(n + 1) * 512], lhsT,
                                    wd_q[kb][:, n * 512:(n + 1) * 512],
                                    start=(kb == 0), stop=(kb == OM // P - 1),
                                )
                        dsb = s2.tile([P, D // 2], F32, tag="c1024b")
                        nc.vector.tensor_copy(dsb[:], pso[:])
                        nc.sync.dma_start(rs_in[t * P:(t + 1) * P, :], dsb[:])
                    nc.gpsimd.collective_compute(
                        "ReduceScatter", ALU.add, replica_groups=rg,
                        ins=[rs_in.opt()], outs=[rs_out.opt()],
                    )

            for lt in range(LT):
                x1r = s2.tile([P, D], F32, tag="x1t", bufs=1)
                nc.sync.dma_start(x1r[:], x1_d[lt * P:(lt + 1) * P, :])
                for oc in range(2):
                    ysb = s2.tile([P, D // 2], F32, tag="c1024")
                    nc.sync.dma_start(
                        ysb[:], rs2_out[oc][lt * P:(lt + 1) * P, :]
                    )
                    ot = s2.tile([P, D // 2], F32, tag="c1024b")
                    nc.vector.scalar_tensor_tensor(
                        ot[:], ysb[:], sc_dl[:, lt:lt + 1],
                        x1r[:, oc * (D // 2):(oc + 1) * (D // 2)],
                        op0=ALU.mult, op1=ALU.add,
                    )
                    nc.sync.dma_start(
                        out_d[lt * P:(lt + 1) * P,
                              oc * (D // 2):(oc + 1) * (D // 2)],
                        ot[:],
                    )

    nc.compile()
    return nc


def _prep_in_maps(inputs):
    x = np.asarray(inputs["x"], np.float32).reshape(R, D)
    wq = np.asarray(inputs["wq"], np.float32)
    wk = np.asarray(inputs["wk"], np.float32)
    wv = np.asarray(inputs["wv"], np.float32)
    wo = np.asarray(inputs["wo"], np.float32)
    wg = np.asarray(inputs["wg"], np.float32)
    wu = np.asarray(inputs["wu"], np.float32)
    wd = np.asarray(inputs["wd"], np.float32)
    n1 = np.asarray(inputs["norm1_w"], np.float32).reshape(1, D)
    n2 = np.asarray(inputs["norm2_w"], np.float32).reshape(1, D)

    ident = np.eye(P, dtype=ml_dtypes.bfloat16)
    iv, jv = np.mgrid[0:P, 0:P]
    causal = np.where(jv <= iv, 0.0, -1e30).astype(np.float32)
    wcnt = np.array(
        [[D * D, D * D, D * D, D * D, MLP * D, MLP * D, D * MLP, 1.0]], np.float64
    )
    wcnt_inv = (1.0 / wcnt).astype(np.float32)

    in_maps = []
    for c in range(NCORES):
        qs = slice(c * OQ, (c + 1) * OQ)
        ms = slice(c * OM, (c + 1) * OM)
        in_maps.append({
            "x_rows": np.ascontiguousarray(x[c * RL:(c + 1) * RL]),
            "wqkvT": np.ascontiguousarray(
                np.concatenate([wq[qs], wk[qs], wv[qs]], 0).T
            ),
            "woT": np.ascontiguousarray(wo[:, qs].T),
            "wguT": np.ascontiguousarray(
                np.concatenate([wg[ms], wu[ms]], 0).T
            ),
            "wdT": np.ascontiguousarray(wd[:, ms].T),
            "norm1_w": n1,
            "norm2_w": n2,
            "ident_b": ident,
            "causal": causal,
            "wcnt_inv": wcnt_inv,
        })
    return in_maps


def kernel(**inputs) -> np.ndarray:
    global _CACHED_NC
    if _CACHED_NC is None:
        _CACHED_NC = build_program()
    nc = _CACHED_NC
    in_maps = _prep_in_maps(inputs)
    res = run_bass_kernel_spmd(nc, in_maps, core_ids=list(range(NCORES)))
    out = np.concatenate([res.results[c]["out"] for c in range(NCORES)], 0)
    return out.reshape(B, S, D).astype(np.float32)



# revision 7
# speedup vs baseline: 1.6352x; 1.6352x over previous
"""BitNet transformer block on 8 Trainium2 NeuronCores — v2.

Strategy (vs v1): weights are ternary-quantized on the HOST (bf16, exact) and
pre-transposed, so the device only does activation quantization, matmuls, and
small collectives. All projections are column-parallel:

  - qkv: each core computes q/k/v for its 2 heads, all tokens.
  - o:   each core computes its 256 output columns, all tokens (contraction
         over the full attn dim via an AllGather of head-sharded outputs).
  - gate/up: core's 1024 MLP cols, all tokens.
  - down: core's 256 output columns, contraction over the full 8192 MLP dim
          via an AllGather of the (quantized, transposed) m activations.

The final output is column-sharded [R, 256] per core; the host concatenates.
No ReduceScatter anywhere — every collective is a small AllGather (bf16) or a
[R,1] AllReduce of scales.  Activations flow in TRANSPOSED layout [feature,
token] so matmul lhsT operands come straight out of AllGather buffers; each
core transposes only its OWN shard, with one batched DMA_TRANSPOSE per
128-row tile instead of per 128x128 block.

Numerics: quantized activations (ints in [-127,127]) and ternary weights are
exact in bf16; matmuls accumulate in fp32 PSUM, so every BitNet matmul is
exact integer arithmetic, matching the fp32 reference fake-quant.
"""

import numpy as np
import ml_dtypes

import concourse.bacc as bacc
import concourse.mybir as mybir
import concourse.tile as tile
from concourse.bass_utils import run_bass_kernel_spmd

F32 = mybir.dt.float32
BF16 = mybir.dt.bfloat16
AF = mybir.ActivationFunctionType
ALU = mybir.AluOpType
AX = mybir.AxisListType

NCORES = 8
B, S, D, H, MLP = 2, 1024, 2048, 16, 8192
HD = 128
R = B * S                 # 2048 token rows total
RL = R // NCORES          # 256 rows per core (phase-1 row shard)
OQ = D // NCORES          # 256 qkv/o cols per core (2 heads)
OM = MLP // NCORES        # 1024 mlp cols per core
P = 128
KT = D // P               # 16 feature chunks
RT = R // P               # 16 token tiles
LT = RL // P              # 2 local row tiles
ST = S // P               # 8 seq tiles per batch
KM = OM // P              # 8 local mlp-dim chunks
MAGIC = 12582912.0        # 1.5 * 2**23: fp32 round-to-nearest-even magic
INV_SQRT_HD = 1.0 / float(np.sqrt(HD))

_CACHED_NC = None


def _quant(nc, sp, src_ap, qscale_ap, out_bf_ap, tag="qtmp"):
    """out_bf = round(src * qscale) as bf16 (qscale: per-partition [P,1])."""
    F = src_ap.shape[1]
    CH = min(F, 1024)
    for c0 in range(0, F, CH):
        tmp = sp.tile([src_ap.shape[0], CH], F32, tag=tag, name=tag)
        nc.vector.tensor_scalar(
            tmp[:], src_ap[:, c0:c0 + CH], qscale_ap, MAGIC,
            op0=ALU.mult, op1=ALU.add,
        )
        nc.scalar.activation(
            out_bf_ap[:, c0:c0 + CH], tmp[:], AF.Copy, bias=-MAGIC, scale=1.0
        )


def _rms_quant_rows(nc, sp, ps_dummy, src_tile, nw_tile, as_out_ap, aq_out_ap):
    """rmsnorm + abs-max + int8-grid quantize for one [128, D] row tile."""
    sqd = ps_dummy.tile([P, D], F32, tag="sqd")
    ssq = sp.tile([P, 1], F32, tag="ssq")
    nc.scalar.activation(sqd[:], src_tile[:], AF.Square, accum_out=ssq[:])
    rms = sp.tile([P, 1], F32, tag="rms")
    nc.vector.tensor_scalar(
        rms[:], ssq[:], 1.0 / D, 1e-6, op0=ALU.mult, op1=ALU.add
    )
    nc.scalar.activation(rms[:], rms[:], AF.Sqrt)
    rinv = sp.tile([P, 1], F32, tag="rinv")
    nc.vector.reciprocal(rinv[:], rms[:])
    nc.vector.tensor_tensor(src_tile[:], src_tile[:], nw_tile[:], op=ALU.mult)
    amax = sp.tile([P, 1], F32, tag="amax")
    nc.vector.tensor_reduce(
        amax[:], src_tile[:], op=ALU.max, axis=AX.X, apply_absolute_value=True
    )
    nc.vector.tensor_scalar(
        as_out_ap, amax[:], rinv[:], 1e-8, op0=ALU.mult, op1=ALU.add
    )
    inva = sp.tile([P, 1], F32, tag="inva")
    nc.vector.reciprocal(inva[:], as_out_ap)
    qs = sp.tile([P, 1], F32, tag="qs")
    nc.vector.tensor_scalar(
        qs[:], inva[:], rinv[:], 127.0, op0=ALU.mult, op1=ALU.mult
    )
    _quant(nc, sp, src_tile[:], qs[:, 0:1], aq_out_ap)


def build_program():
    nc = bacc.Bacc(
        "TRN2",
        target_bir_lowering=False,
        debug=False,
        enable_asserts=True,
        num_devices=NCORES,
    )
    rg = [list(range(NCORES))]

    # ---------------- I/O ----------------
    x_rows = nc.dram_tensor("x_rows", [RL, D], F32, kind="ExternalInput").ap()
    x_cols = nc.dram_tensor("x_cols", [R, OQ], F32, kind="ExternalInput").ap()
    wqkv_b = nc.dram_tensor("wqkv_b", [D, 3 * OQ], BF16, kind="ExternalInput").ap()
    wo_b = nc.dram_tensor("wo_b", [D, OQ], BF16, kind="ExternalInput").ap()
    wgu_b = nc.dram_tensor("wgu_b", [D, 2 * OM], BF16, kind="ExternalInput").ap()
    wd_b = nc.dram_tensor("wd_b", [MLP, OQ], BF16, kind="ExternalInput").ap()
    norm1_w = nc.dram_tensor("norm1_w", [1, D], F32, kind="ExternalInput").ap()
    norm2c_w = nc.dram_tensor("norm2c_w", [1, OQ], F32, kind="ExternalInput").ap()
    csc = nc.dram_tensor("csc", [1, 8], F32, kind="ExternalInput").ap()
    causal = nc.dram_tensor("causal", [P, P], F32, kind="ExternalInput").ap()
    out_d = nc.dram_tensor("out", [R, OQ], F32, kind="ExternalOutput").ap()

    with tile.TileContext(nc) as tc, \
         tc.tile_pool(name="persist", bufs=1) as pp, \
         tc.tile_pool(name="dram", bufs=1, space="DRAM") as dp:

        # ---------------- collective DRAM buffers ----------------
        ag1_in = dp.tile([RL, D], BF16, tag="ag1_in")
        ag1_out = dp.tile([R, D], BF16, tag="ag1_out", addr_space="Shared")
        ag1s_in = dp.tile([RL, 1], F32, tag="ag1s_in")
        ag1s_out = dp.tile([R, 1], F32, tag="ag1s_out", addr_space="Shared")
        aso_in = dp.tile([R, 1], F32, tag="aso_in")
        aso_g = dp.tile([R, 1], F32, tag="aso_g", addr_space="Shared")
        ago_in = dp.tile([OQ, R], BF16, tag="ago_in")
        ago_out = dp.tile([D, R], BF16, tag="ago_out", addr_space="Shared")
        ssq_in = dp.tile([R, 1], F32, tag="ssq_in")
        ssq_g = dp.tile([R, 1], F32, tag="ssq_g", addr_space="Shared")
        am2_in = dp.tile([R, 1], F32, tag="am2_in")
        am2_g = dp.tile([R, 1], F32, tag="am2_g", addr_space="Shared")
        ag2_in = dp.tile([OQ, R], BF16, tag="ag2_in")
        ag2_out = dp.tile([D, R], BF16, tag="ag2_out", addr_space="Shared")
        asm_in = [dp.tile([4 * P, 1], F32, tag=f"asmi{g}", name=f"asmi{g}")
                  for g in range(4)]
        asm_g = [dp.tile([4 * P, 1], F32, tag=f"asmg{g}", name=f"asmg{g}", addr_space="Shared")
                 for g in range(4)]
        agm_in = [dp.tile([OM, S], BF16, tag=f"agmi{h}", name=f"agmi{h}")
                  for h in range(2)]
        agm_out = [dp.tile([MLP, S], BF16, tag=f"agmo{h}", name=f"agmo{h}", addr_space="Shared")
                   for h in range(2)]

        # ---------------- persistent small tiles ----------------
        mask = pp.tile([P, P], F32, tag="mask")
        nc.sync.dma_start(mask[:], causal)
        cscr = pp.tile([1, 8], F32, tag="cscr")
        nc.sync.dma_start(cscr[:], csc)
        cb = pp.tile([P, 8], F32, tag="cb")
        nc.gpsimd.partition_broadcast(cb[:], cscr[0:1, :])
        # cb columns: 0=c_qk 1=c_v 2=c_o 3=c_g 4=c_u 5=c_d

        as1g = pp.tile([P, RT], F32, tag="as1g")
        cq = pp.tile([P, RT], F32, tag="cq")
        scv = pp.tile([P, RT], F32, tag="scv")
        asog = pp.tile([P, RT], F32, tag="asog")
        qso = pp.tile([P, RT], F32, tag="qso")
        sc_o = pp.tile([P, RT], F32, tag="sc_o")
        ssqg = pp.tile([P, RT], F32, tag="ssqg")
        rinv2 = pp.tile([P, RT], F32, tag="rinv2")
        am2g = pp.tile([P, RT], F32, tag="am2g")
        as2 = pp.tile([P, RT], F32, tag="as2")
        f2 = pp.tile([P, RT], F32, tag="f2")
        sc_g = pp.tile([P, RT], F32, tag="sc_g")
        sc_u = pp.tile([P, RT], F32, tag="sc_u")
        asmg_t = pp.tile([P, RT], F32, tag="asmg_t")
        qsm = pp.tile([P, RT], F32, tag="qsm")
        sc_d = pp.tile([P, RT], F32, tag="sc_d")
        nw2c = pp.tile([P, OQ], F32, tag="nw2c")
        nw2r = pp.tile([1, OQ], F32, tag="nw2r")
        nc.sync.dma_start(nw2r[:], norm2c_w)
        nc.gpsimd.partition_broadcast(nw2c[:], nw2r[0:1, :])
        x1 = [pp.tile([P, OQ], F32, tag=f"x1_{t}", name=f"x1_{t}")
              for t in range(RT)]

        # ---- Phase 1: local rmsnorm1 + quant + transpose + AG ----
        with tc.tile_pool(name="p1s", bufs=2) as s1, \
             tc.tile_pool(name="ps0", bufs=1, space="PSUM") as ps0:
            nw1 = s1.tile([P, D], F32, tag="nw1", bufs=1)
            nw1r = s1.tile([1, D], F32, tag="nw1r", bufs=1)
            nc.sync.dma_start(nw1r[:], norm1_w)
            nc.gpsimd.partition_broadcast(nw1[:], nw1r[0:1, :])
            for lt in range(LT):
                xt = s1.tile([P, D], F32, tag="xt")
                nc.sync.dma_start(xt[:], x_rows[lt * P:(lt + 1) * P, :])
                as_l = s1.tile([P, 1], F32, tag="as_l")
                aq = s1.tile([P, D], BF16, tag="aq")
                _rms_quant_rows(nc, s1, ps0, xt, nw1, as_l[:, 0:1], aq[:])
                nc.sync.dma_start(ag1s_in[lt * P:(lt + 1) * P, :], as_l[:])
                tq1 = s1.tile([P, KT, P], BF16, tag="tq1")
                nc.sync.dma_start(tq1[:], aq[:], transpose=True)
                nc.sync.dma_start(
                    ag1_in[lt * P:(lt + 1) * P, :]
                    .rearrange("p (kb q) -> p kb q", q=P),
                    tq1[:],
                )
        nc.gpsimd.collective_compute(
            "AllGather", ALU.bypass, replica_groups=rg,
            ins=[ag1_in.opt()], outs=[ag1_out.opt()],
        )
        nc.gpsimd.collective_compute(
            "AllGather", ALU.bypass, replica_groups=rg,
            ins=[ag1s_in.opt()], outs=[ag1s_out.opt()],
        )

        # =========================================================
        # Pool B spans phases 2-4 (qkT/v live through attention; wo
        # through o-proj).  Pool A (big transposed acts + qkv weights)
        # closes after attention so phase 4 can reuse its space.
        # =========================================================
        with tc.tile_pool(name="pB", bufs=1) as pB:
            qkT_sb = [pB.tile([P, R], BF16, tag=f"qkT{i}", name=f"qkT{i}")
                      for i in range(4)]
            v_sb = [pB.tile([P, 2 * P], BF16, tag=f"v{t}", name=f"v{t}")
                    for t in range(RT)]
            wo_sb = [pB.tile([P, OQ], BF16, tag=f"wo{k}", name=f"wo{k}")
                     for k in range(KT)]

            with tc.tile_pool(name="pA", bufs=1) as pA:
                wqkv_sb = [pA.tile([P, 3 * OQ], BF16, tag=f"wqkv{k}",
                                   name=f"wqkv{k}") for k in range(KT)]
                for k in range(KT):
                    nc.scalar.dma_start(
                        wqkv_sb[k][:], wqkv_b[k * P:(k + 1) * P, :]
                    )
                # scale prep
                nc.sync.dma_start(
                    as1g[:], ag1s_out.rearrange("(t p) o -> p (t o)", p=P)
                )
                nc.vector.tensor_scalar(
                    cq[:], as1g[:], cb[:, 0:1], None, op0=ALU.mult
                )
                nc.vector.tensor_scalar(
                    scv[:], as1g[:], cb[:, 1:2], None, op0=ALU.mult
                )
                dk_row = pA.tile([1, R], F32, tag="dk_row")
                nc.sync.dma_start(dk_row[:], ag1s_out.rearrange("r o -> o r"))
                dkb = pA.tile([P, R], F32, tag="dkb")
                nc.gpsimd.partition_broadcast(dkb[:], dk_row[0:1, :])

                # ---- Phase 2: QKV ----
                aT = pA.tile([P, KT, R], BF16, tag="aT")
                for t in range(RT):
                    nc.sync.dma_start(
                        aT[:, :, t * P:(t + 1) * P],
                        ag1_out[t * P:(t + 1) * P, :]
                        .rearrange("p (kb q) -> p kb q", q=P),
                    )
                with tc.tile_pool(name="ps2", bufs=2, space="PSUM") as ps2:
                    # order q0, k0, q1, k1 so head-0 attention starts early
                    for i in (0, 2, 1, 3):
                        wlo = (i % 2) * P + (0 if i < 2 else OQ)
                        for tc4 in range(4):
                            psq = ps2.tile([P, 512], F32, tag="psq")
                            for kb in range(KT):
                                nc.tensor.matmul(
                                    psq[:], wqkv_sb[kb][:, wlo:wlo + P],
                                    aT[:, kb, tc4 * 512:(tc4 + 1) * 512],
                                    start=(kb == 0), stop=(kb == KT - 1),
                                )
                            nc.vector.tensor_copy(
                                qkT_sb[i][:, tc4 * 512:(tc4 + 1) * 512],
                                psq[:],
                            )
                    for t in range(RT):
                        psv = ps2.tile([P, 2 * P], F32, tag="psv")
                        for kb in range(KT):
                            nc.tensor.matmul(
                                psv[:], aT[:, kb, t * P:(t + 1) * P],
                                wqkv_sb[kb][:, 2 * OQ:3 * OQ],
                                start=(kb == 0), stop=(kb == KT - 1),
                            )
                        nc.vector.tensor_scalar(
                            v_sb[t][:], psv[:], scv[:, t:t + 1], None,
                            op0=ALU.mult,
                        )

                # o-proj weights prefetch during attention
                for k in range(KT):
                    nc.scalar.dma_start(wo_sb[k][:], wo_b[k * P:(k + 1) * P, :])

                # ---- Phase 3: attention ----
                att_sb = [pA.tile([P, 2 * P], F32, tag=f"att{t}",
                                  name=f"att{t}") for t in range(RT)]
                aso_sb = pA.tile([P, RT], F32, tag="aso_sb")
                with tc.tile_pool(name="pAt", bufs=2) as sat, \
                     tc.tile_pool(name="ps3s", bufs=2, space="PSUM") as ps3s, \
                     tc.tile_pool(name="ps3a", bufs=4, space="PSUM") as ps3a:
                    for b in range(B):
                        for i in range(ST):
                            tg = b * ST + i
                            L = (i + 1) * P
                            for hl in range(2):
                                q_i, k_i = hl, 2 + hl
                                Spp = ps3s.tile([P, S], F32, tag="Spp")
                                lhsT = qkT_sb[q_i][:, tg * P:(tg + 1) * P]
                                for jc in range((L + 511) // 512):
                                    n0 = jc * 512
                                    n1 = min(L, n0 + 512)
                                    nc.tensor.matmul(
                                        Spp[:, n0:n1], lhsT,
                                        qkT_sb[k_i][:, b * S + n0:b * S + n1],
                                        start=True, stop=True,
                                    )
                                S1 = sat.tile([P, S], F32, tag="S1")
                                nc.vector.scalar_tensor_tensor(
                                    S1[:, 0:L], Spp[:, 0:L], cq[:, tg:tg + 1],
                                    dkb[:, b * S:b * S + L],
                                    op0=ALU.mult, op1=ALU.mult,
                                )
                                nc.vector.tensor_tensor(
                                    S1[:, i * P:L], S1[:, i * P:L], mask[:],
                                    op=ALU.add,
                                )
                                negmx = sat.tile([P, 1], F32, tag="negmx")
                                nc.vector.tensor_reduce(
                                    negmx[:], S1[:, 0:L], op=ALU.max,
                                    axis=AX.X, negate=True,
                                )
                                esum = sat.tile([P, 1], F32, tag="esum")
                                nc.scalar.activation(
                                    S1[:, 0:L], S1[:, 0:L], AF.Exp,
                                    bias=negmx[:, 0:1], scale=1.0,
                                    accum_out=esum[:],
                                )
                                erec = sat.tile([P, 1], F32, tag="erec")
                                nc.vector.reciprocal(erec[:], esum[:])
                                Pb = sat.tile([P, S], BF16, tag="Pb")
                                nc.vector.tensor_scalar(
                                    Pb[:, 0:L], S1[:, 0:L], erec[:, 0:1],
                                    None, op0=ALU.mult,
                                )
                                pts = sat.tile([P, ST, P], BF16, tag="pts")
                                nc.sync.dma_start(
                                    pts[:, 0:i + 1, :], Pb[:, 0:L],
                                    transpose=True,
                                )
                                att = ps3a.tile([P, P], F32, tag="att")
                                for j in range(i + 1):
                                    nc.tensor.matmul(
                                        att[:], pts[:, j, :],
                                        v_sb[b * ST + j]
                                        [:, hl * P:(hl + 1) * P],
                                        start=(j == 0), stop=(j == i),
                                    )
                                nc.vector.tensor_copy(
                                    att_sb[tg][:, hl * P:(hl + 1) * P],
                                    att[:],
                                )
                            nc.vector.tensor_reduce(
                                aso_sb[:, tg:tg + 1], att_sb[tg][:],
                                op=ALU.max, axis=AX.X,
                                apply_absolute_value=True,
                            )
                # global attn-out scale
                nc.sync.dma_start(
                    aso_in.rearrange("(t p) o -> p (t o)", p=P), aso_sb[:]
                )
                nc.gpsimd.collective_compute(
                    "AllReduce", ALU.max, replica_groups=rg,
                    ins=[aso_in.opt()], outs=[aso_g.opt()],
                )
                nc.sync.dma_start(
                    asog[:], aso_g.rearrange("(t p) o -> p (t o)", p=P)
                )
                nc.vector.tensor_scalar(
                    asog[:], asog[:], 1e-8, None, op0=ALU.add
                )
                nc.vector.reciprocal(qso[:], asog[:])
                nc.vector.tensor_scalar(
                    qso[:], qso[:], 127.0, None, op0=ALU.mult
                )
                nc.vector.tensor_scalar(
                    sc_o[:], asog[:], cb[:, 2:3], None, op0=ALU.mult
                )
                # quantize + transpose attn-out into ago_in
                with tc.tile_pool(name="pqa", bufs=2) as sqa:
                    for t in range(RT):
                        a_qo = sqa.tile([P, 2 * P], BF16, tag="a_qo")
                        _quant(nc, sqa, att_sb[t][:], qso[:, t:t + 1],
                               a_qo[:], tag="qat")
                        tqa = sqa.tile([P, 2, P], BF16, tag="tqa")
                        nc.sync.dma_start(tqa[:], a_qo[:], transpose=True)
                        nc.scalar.dma_start(
                            ago_in.rearrange("(kb p) r -> p kb r", p=P)
                            [:, :, t * P:(t + 1) * P],
                            tqa[:],
                        )
                nc.gpsimd.collective_compute(
                    "AllGather", ALU.bypass, replica_groups=rg,
                    ins=[ago_in.opt()], outs=[ago_out.opt()],
                )
            # pool A frees (aT, wqkv, dkb, att_sb...)

            # ---- Phase 4: o-proj + residual + rmsnorm2 + AG2 ----
            with tc.tile_pool(name="pO", bufs=1) as pO:
                aoT = pO.tile([P, KT, R], BF16, tag="aoT")
                for t in range(RT):
                    nc.sync.dma_start(
                        aoT[:, :, t * P:(t + 1) * P],
                        ago_out.rearrange("(kb p) r -> p kb r", p=P)
                        [:, :, t * P:(t + 1) * P],
                    )
                xc = [pO.tile([P, OQ], F32, tag=f"xc{t}", name=f"xc{t}")
                      for t in range(RT)]
                for t in range(RT):
                    nc.scalar.dma_start(xc[t][:], x_cols[t * P:(t + 1) * P, :])

                with tc.tile_pool(name="psO", bufs=4, space="PSUM") as psO, \
                     tc.tile_pool(name="p4s", bufs=2) as s4:
                    for t in range(RT):
                        pso = psO.tile([P, OQ], F32, tag="pso")
                        for kb in range(KT):
                            nc.tensor.matmul(
                                pso[:], aoT[:, kb, t * P:(t + 1) * P],
                                wo_sb[kb][:],
                                start=(kb == 0), stop=(kb == KT - 1),
                            )
                        nc.vector.scalar_tensor_tensor(
                            x1[t][:], pso[:], sc_o[:, t:t + 1], xc[t][:],
                            op0=ALU.mult, op1=ALU.add,
                        )
                        # norm2 partials: ssq(x1) and max|x1*nw2|
                        sq = s4.tile([P, OQ], F32, tag="sq")
                        ssq_l = s4.tile([P, 1], F32, tag="ssq_l")
                        nc.scalar.activation(
                            sq[:], x1[t][:], AF.Square, accum_out=ssq_l[:]
                        )
                        hn = s4.tile([P, OQ], F32, tag="hn")
                        nc.vector.tensor_tensor(
                            hn[:], x1[t][:], nw2c[:], op=ALU.mult
                        )
                        am_l = s4.tile([P, 1], F32, tag="am_l")
                        nc.vector.tensor_reduce(
                            am_l[:], hn[:], op=ALU.max, axis=AX.X,
                            apply_absolute_value=True,
                        )
                        nc.sync.dma_start(
                            ssq_in[t * P:(t + 1) * P, :], ssq_l[:]
                        )
                        nc.sync.dma_start(
                            am2_in[t * P:(t + 1) * P, :], am_l[:]
                        )
                nc.gpsimd.collective_compute(
                    "AllReduce", ALU.add, replica_groups=rg,
                    ins=[ssq_in.opt()], outs=[ssq_g.opt()],
                )
                nc.gpsimd.collective_compute(
                    "AllReduce", ALU.max, replica_groups=rg,
                    ins=[am2_in.opt()], outs=[am2_g.opt()],
                )
                # rms2: rinv2 = 1/sqrt(ssq/D + 1e-6); as2 = amax*rinv2 + 1e-8
                nc.sync.dma_start(
                    ssqg[:], ssq_g.rearrange("(t p) o -> p (t o)", p=P)
                )
                nc.sync.dma_start(
                    am2g[:], am2_g.rearrange("(t p) o -> p (t o)", p=P)
                )
                nc.vector.tensor_scalar(
                    rinv2[:], ssqg[:], 1.0 / D, 1e-6,
                    op0=ALU.mult, op1=ALU.add,
                )
                nc.scalar.activation(rinv2[:], rinv2[:], AF.Sqrt)
                nc.vector.reciprocal(rinv2[:], rinv2[:])
                nc.vector.tensor_tensor(
                    as2[:], am2g[:], rinv2[:], op=ALU.mult
                )
                nc.vector.tensor_scalar(
                    as2[:], as2[:], 1e-8, None, op0=ALU.add
                )
                nc.vector.reciprocal(f2[:], as2[:])
                nc.vector.tensor_tensor(f2[:], f2[:], rinv2[:], op=ALU.mult)
                nc.vector.tensor_scalar(
                    f2[:], f2[:], 127.0, None, op0=ALU.mult
                )
                nc.vector.tensor_scalar(
                    sc_g[:], as2[:], cb[:, 3:4], None, op0=ALU.mult
                )
                nc.vector.tensor_scalar(
                    sc_u[:], as2[:], cb[:, 4:5], None, op0=ALU.mult
                )
                # quantize x1 cols + transpose into ag2_in
                with tc.tile_pool(name="pq2", bufs=2) as sq2:
                    for t in range(RT):
                        tmp = sq2.tile([P, OQ], F32, tag="q2tmp")
                        nc.vector.scalar_tensor_tensor(
                            tmp[:], x1[t][:], f2[:, t:t + 1], nw2c[:],
                            op0=ALU.mult, op1=ALU.mult,
                        )
                        nc.vector.tensor_scalar(
                            tmp[:], tmp[:], MAGIC, None, op0=ALU.add
                        )
                        a_q2 = sq2.tile([P, OQ], BF16, tag="a_q2")
                        nc.scalar.activation(
                            a_q2[:], tmp[:], AF.Copy, bias=-MAGIC, scale=1.0
                        )
                        tq2 = sq2.tile([P, 2, P], BF16, tag="tq2")
                        nc.sync.dma_start(tq2[:], a_q2[:], transpose=True)
                        nc.scalar.dma_start(
                            ag2_in.rearrange("(kb p) r -> p kb r", p=P)
                            [:, :, t * P:(t + 1) * P],
                            tq2[:],
                        )
                nc.gpsimd.collective_compute(
                    "AllGather", ALU.bypass, replica_groups=rg,
                    ins=[ag2_in.opt()], outs=[ag2_out.opt()],
                )
        # pool B frees

        # ---- Phase 5: MLP gate/up + m quant + AG(mT) ----
        with tc.tile_pool(name="pM", bufs=1) as pM:
            wgu_sb = [pM.tile([P, 2 * OM], BF16, tag=f"wgu{k}", name=f"wgu{k}")
                      for k in range(KT)]
            for k in range(KT):
                nc.scalar.dma_start(wgu_sb[k][:], wgu_b[k * P:(k + 1) * P, :])
            a2T = pM.tile([P, KT, R], BF16, tag="a2T")
            for t in range(RT):
                nc.sync.dma_start(
                    a2T[:, :, t * P:(t + 1) * P],
                    ag2_out.rearrange("(kb p) r -> p kb r", p=P)
                    [:, :, t * P:(t + 1) * P],
                )
            asm_sb = pM.tile([P, RT], F32, tag="asm_sb")
            with tc.tile_pool(name="p5s", bufs=2) as s5, \
                 tc.tile_pool(name="ps5", bufs=2, space="PSUM") as ps5:
                for g in range(4):
                    m_tiles = {}
                    for tl in range(4):
                        t = g * 4 + tl
                        psg = ps5.tile([P, OM], F32, tag="psg")
                        psu = ps5.tile([P, OM], F32, tag="psu")
                        for kb in range(KT):
                            for n in range(2):
                                nc.tensor.matmul(
                                    psg[:, n * 512:(n + 1) * 512],
                                    a2T[:, kb, t * P:(t + 1) * P],
                                    wgu_sb[kb][:, n * 512:(n + 1) * 512],
                                    start=(kb == 0), stop=(kb == KT - 1),
                                )
                        for kb in range(KT):
                            for n in range(2):
                                nc.tensor.matmul(
                                    psu[:, n * 512:(n + 1) * 512],
                                    a2T[:, kb, t * P:(t + 1) * P],
                                    wgu_sb[kb]
                                    [:, OM + n * 512:OM + (n + 1) * 512],
                                    start=(kb == 0), stop=(kb == KT - 1),
                                )
                        sig = s5.tile([P, OM], F32, tag="sig")
                        nc.scalar.activation(
                            sig[:], psg[:], AF.Sigmoid,
                            scale=sc_g[:, t:t + 1],
                        )
                        sgl = s5.tile([P, OM], F32, tag="sgl")
                        nc.vector.scalar_tensor_tensor(
                            sgl[:], psg[:], sc_g[:, t:t + 1], sig[:],
                            op0=ALU.mult, op1=ALU.mult,
                        )
                        mt = s5.tile([P, OM], F32, tag=f"m{tl}",
                                     name=f"m{tl}", bufs=1)
                        m_tiles[tl] = mt
                        nc.vector.scalar_tensor_tensor(
                            mt[:], psu[:], sc_u[:, t:t + 1], sgl[:],
                            op0=ALU.mult, op1=ALU.mult,
                        )
                        nc.vector.tensor_reduce(
                            asm_sb[:, t:t + 1], mt[:], op=ALU.max, axis=AX.X,
                            apply_absolute_value=True,
                        )
                    nc.sync.dma_start(
                        asm_in[g].rearrange("(t p) o -> p (t o)", p=P),
                        asm_sb[:, g * 4:(g + 1) * 4],
                    )
                    nc.gpsimd.collective_compute(
                        "AllReduce", ALU.max, replica_groups=rg,
                        ins=[asm_in[g].opt()], outs=[asm_g[g].opt()],
                    )
                    gs = slice(g * 4, (g + 1) * 4)
                    nc.sync.dma_start(
                        asmg_t[:, gs],
                        asm_g[g].rearrange("(t p) o -> p (t o)", p=P),
                    )
                    nc.vector.tensor_scalar(
                        asmg_t[:, gs], asmg_t[:, gs], 1e-8, None, op0=ALU.add
                    )
                    nc.vector.reciprocal(qsm[:, gs], asmg_t[:, gs])
                    nc.vector.tensor_scalar(
                        qsm[:, gs], qsm[:, gs], 127.0, None, op0=ALU.mult
                    )
                    nc.vector.tensor_scalar(
                        sc_d[:, gs], asmg_t[:, gs], cb[:, 5:6], None,
                        op0=ALU.mult,
                    )
                    for tl in range(4):
                        t = g * 4 + tl
                        m_q = s5.tile([P, OM], BF16, tag="m_q")
                        _quant(nc, s5, m_tiles[tl][:], qsm[:, t:t + 1],
                               m_q[:], tag="qm")
                        tqm = s5.tile([P, KM, P], BF16, tag="tqm")
                        nc.sync.dma_start(tqm[:], m_q[:], transpose=True)
                        h = t // ST
                        nc.scalar.dma_start(
                            agm_in[h].rearrange("(kb p) r -> p kb r", p=P)
                            [:, :, (t - h * ST) * P:(t - h * ST + 1) * P],
                            tqm[:],
                        )
                    if g % 2 == 1:
                        h = g // 2
                        nc.gpsimd.collective_compute(
                            "AllGather", ALU.bypass, replica_groups=rg,
                            ins=[agm_in[h].opt()], outs=[agm_out[h].opt()],
                        )

        # ---- Phase 6: down (column-parallel, full-MLP contraction) ----
        with tc.tile_pool(name="pD", bufs=1) as pD, \
             tc.tile_pool(name="p6s", bufs=3) as s6, \
             tc.tile_pool(name="ps6", bufs=1, space="PSUM") as ps6:
            wd_sb = [pD.tile([P, OQ], BF16, tag=f"wd{k}", name=f"wd{k}")
                     for k in range(MLP // P)]
            for k in range(MLP // P):
                nc.scalar.dma_start(wd_sb[k][:], wd_b[k * P:(k + 1) * P, :])
            for hh in range(2):
                psd = [ps6.tile([P, OQ], F32, tag=f"psd{tl}", name=f"psd{tl}")
                       for tl in range(ST)]
                for kb in range(MLP // P):
                    mt_l = s6.tile([P, S], BF16, tag="mt_l")
                    nc.scalar.dma_start(
                        mt_l[:], agm_out[hh][kb * P:(kb + 1) * P, :]
                    )
                    for tl in range(ST):
                        nc.tensor.matmul(
                            psd[tl][:], mt_l[:, tl * P:(tl + 1) * P],
                            wd_sb[kb][:],
                            start=(kb == 0), stop=(kb == MLP // P - 1),
                        )
                for tl in range(ST):
                    t = hh * ST + tl
                    ot = s6.tile([P, OQ], F32, tag="ot")
                    nc.vector.scalar_tensor_tensor(
                        ot[:], psd[tl][:], sc_d[:, t:t + 1], x1[t][:],
                        op0=ALU.mult, op1=ALU.add,
                    )
                    nc.sync.dma_start(out_d[t * P:(t + 1) * P, :], ot[:])

    nc.compile()
    return nc


def _prep_in_maps(inputs):
    x = np.asarray(inputs["x"], np.float32).reshape(R, D)
    wq = np.asarray(inputs["wq"], np.float32)
    wk = np.asarray(inputs["wk"], np.float32)
    wv = np.asarray(inputs["wv"], np.float32)
    wo = np.asarray(inputs["wo"], np.float32)
    wg = np.asarray(inputs["wg"], np.float32)
    wu = np.asarray(inputs["wu"], np.float32)
    wd = np.asarray(inputs["wd"], np.float32)
    n1 = np.asarray(inputs["norm1_w"], np.float32).reshape(1, D)
    n2 = np.asarray(inputs["norm2_w"], np.float32).reshape(1, D)

    def wscale(w):
        return float(np.abs(w.astype(np.float64)).mean()) + 1e-8

    def tern(w, ws):
        return np.clip(np.round(w / np.float32(ws)), -1.0, 1.0) \
            .astype(ml_dtypes.bfloat16)

    ws_q, ws_k, ws_v = wscale(wq), wscale(wk), wscale(wv)
    ws_o, ws_g, ws_u, ws_d = wscale(wo), wscale(wg), wscale(wu), wscale(wd)
    wq_t = tern(wq, ws_q)
    wk_t = tern(wk, ws_k)
    wv_t = tern(wv, ws_v)
    wo_t = tern(wo, ws_o)
    wg_t = tern(wg, ws_g)
    wu_t = tern(wu, ws_u)
    wd_t = tern(wd, ws_d)

    csc = np.array([[
        ws_q * ws_k * INV_SQRT_HD / (127.0 * 127.0),
        ws_v / 127.0, ws_o / 127.0, ws_g / 127.0, ws_u / 127.0,
        ws_d / 127.0, 0.0, 0.0,
    ]], np.float32)
    iv, jv = np.mgrid[0:P, 0:P]
    causal = np.where(jv <= iv, 0.0, -1e30).astype(np.float32)

    in_maps = []
    for c in range(NCORES):
        qs = slice(c * OQ, (c + 1) * OQ)
        ms = slice(c * OM, (c + 1) * OM)
        in_maps.append({
            "x_rows": np.ascontiguousarray(x[c * RL:(c + 1) * RL]),
            "x_cols": np.ascontiguousarray(x[:, qs]),
            "wqkv_b": np.ascontiguousarray(
                np.concatenate([wq_t[qs], wk_t[qs], wv_t[qs]], 0).T
            ),
            "wo_b": np.ascontiguousarray(wo_t[qs].T),
            "wgu_b": np.ascontiguousarray(
                np.concatenate([wg_t[ms], wu_t[ms]], 0).T
            ),
            "wd_b": np.ascontiguousarray(wd_t[qs].T),
            "norm1_w": n1,
            "norm2c_w": np.ascontiguousarray(n2[:, qs]),
            "csc": csc,
            "causal": causal,
        })
    return in_maps


def _assemble(results) -> np.ndarray:
    out = np.empty((R, D), np.float32)
    for c in range(NCORES):
        out[:, c * OQ:(c + 1) * OQ] = results[c]["out"]
    return out.reshape(B, S, D)


def kernel(**inputs) -> np.ndarray:
    global _CACHED_NC
    if _CACHED_NC is None:
        _CACHED_NC = build_program()
    nc = _CACHED_NC
    in_maps = _prep_in_maps(inputs)
    res = run_bass_kernel_spmd(nc, in_maps, core_ids=list(range(NCORES)))
    return _assemble(res.results).astype(np.float32)
